# revision 3
# baseline (speedup 1.0000x reference)
"""Trainium2 Bass kernel: two-branch GCN embedding (DMGI-style).

Computation per branch (see reference):
    pos_h = relu(gcn_conv(x * mask_pos, W, b, edge_index))
    neg_h = relu(gcn_conv((x * mask_neg)[perm], W, b, edge_index))
    summary = mean(pos_h, axis=0)

Strategy (8 NeuronCores, SPMD, no collectives):
  - Cores 0-3 run branch 1, cores 4-7 branch 2; each core owns a quarter of
    the destination nodes of its branch.
  - Host prep: transpose/cast inputs to bf16 (feature-major), absorb the neg
    permutation into a permuted copy of x / mask_neg, bucket edges
    (+self-loops) by destination tile, compute symmetric norms, and build
    per-128-edge-chunk one-hot matrices M (M[e, d] = norm_e iff dst_e == d).
  - Device phase 1: y = [(x.mask_pos) @ W | (x_perm.mask_neg_perm) @ W] for
    all 50k nodes (replicated per core), stored row-major bf16 in DRAM.
  - Device phase 2: for each destination tile, dma_gather the y rows of its
    incoming edges (512B rows) and accumulate PSUM[dst,feat] via
    out += M_chunk^T @ Y_chunk on the TensorEngine; epilogue adds bias,
    applies relu, stores, and accumulates the summary.
  - Host unshard: reassemble rows, sum partial summaries.

dma_gather indices are int16, so each tile's edges are split into
src < 32768 ("lo") and src >= 32768 ("hi", gathered from a base-offset view).
Per-position chunk counts are data-dependent but must be uniform across cores
(SPMD shares one NEFF): counts are maxed across cores after sorting each
core's tiles by edge count (bin-packing keeps the padding waste ~3%).
"""

import numpy as np
import ml_dtypes

import concourse.bacc as bacc
import concourse.mybir as mybir
import concourse.tile as tile
from concourse.bass_utils import run_bass_kernel_spmd

# ---- hardcoded problem constants ----
N = 50000
D = 128
P = 128
FEAT = 256                       # fused pos|neg row width
NN = 50048                       # N padded to 391*128
NCORES = 8
CORES_PER_BRANCH = 4
DST_PER_CORE = N // CORES_PER_BRANCH      # 12500
TILES = (DST_PER_CORE + P - 1) // P       # 98
PAD_ROWS = TILES * P - DST_PER_CORE       # 44
LO_LIMIT = 32768
S_SUPER = 4096
GROUP_TILES = 4
MAX_GATHER_CH = 32               # chunks (x128 idx) per dma_gather call

bf16 = mybir.dt.bfloat16
f32 = mybir.dt.float32
i16 = mybir.dt.int16
nbf = ml_dtypes.bfloat16


def _wrap_idx(a):
    """[n] int16 -> [128, n//16] wrapped (j at [j%16, j//16]) replicated x8."""
    return np.tile(a.reshape(-1, 16).T, (8, 1)).copy()


def _prep(inputs):
    x = np.asarray(inputs["x"], np.float32)
    branches = []
    for bi in (1, 2):
        W = np.asarray(inputs[f"W{bi}"], np.float32)
        b = np.asarray(inputs[f"b{bi}"], np.float32)
        mp = np.asarray(inputs[f"mask_pos{bi}"], np.float32)
        mn = np.asarray(inputs[f"mask_neg{bi}"], np.float32)
        ei = np.asarray(inputs[f"edge_index{bi}"])
        pm = np.asarray(inputs[f"perm{bi}"]).astype(np.int64)
        src = np.concatenate([ei[0].astype(np.int64), np.arange(N, dtype=np.int64)])
        dst = np.concatenate([ei[1].astype(np.int64), np.arange(N, dtype=np.int64)])
        deg = np.bincount(dst, minlength=N).astype(np.float64)
        dinv = 1.0 / np.sqrt(np.maximum(deg, 1.0))
        w = (dinv[src] * dinv[dst]).astype(np.float32)

        xT = np.zeros((P, NN), nbf)
        xT[:, :N] = x.T
        xpT = np.zeros((P, NN), nbf)
        xpT[:, :N] = x[pm].T
        mpT = np.zeros((P, NN), nbf)
        mpT[:, :N] = mp.T
        mnT = np.zeros((P, NN), nbf)
        mnT[:, :N] = mn[pm].T
        bias = np.broadcast_to(
            np.concatenate([b, b]).astype(np.float32), (P, FEAT)).copy()
        branches.append(dict(
            W=np.ascontiguousarray(W.astype(nbf)), b=b, bias=bias,
            xT=xT, xpT=xpT, mpT=mpT, mnT=mnT, src=src, dst=dst, w=w))

    # per-core edge bucketing
    percore = []
    for core in range(NCORES):
        br = branches[core // CORES_PER_BRANCH]
        base = (core % CORES_PER_BRANCH) * DST_PER_CORE
        sel = (br["dst"] >= base) & (br["dst"] < base + DST_PER_CORE)
        s = br["src"][sel]
        dl = br["dst"][sel] - base
        ww = br["w"][sel]
        t = dl >> 7
        d128 = dl & 127
        hi = (s >= LO_LIMIT).astype(np.int64)
        key = t * 2 + hi
        o = np.argsort(key, kind="stable")
        s, d128, ww, key = s[o], d128[o], ww[o], key[o]
        cnt = np.bincount(key, minlength=TILES * 2)
        seg = np.concatenate([[0], np.cumsum(cnt)])
        locnt, hicnt = cnt[0::2], cnt[1::2]
        order = np.argsort(-locnt, kind="stable")      # position -> tile id
        percore.append(dict(s=s, d128=d128, ww=ww, seg=seg,
                            locnt=locnt, hicnt=hicnt, order=order))

    # global per-position chunk counts (uniform across cores)
    TLO = np.zeros(TILES, np.int64)
    THI = np.zeros(TILES, np.int64)
    for pc in percore:
        TLO = np.maximum(TLO, -(-pc["locnt"][pc["order"]] // P))
        THI = np.maximum(THI, -(-pc["hicnt"][pc["order"]] // P))
    TLO = TLO.astype(np.int64)
    THI = THI.astype(np.int64)
    LOCH, HICH = int(TLO.sum()), int(THI.sum())
    CHT = LOCH + HICH
    lob = np.concatenate([[0], np.cumsum(TLO)])        # lo chunk base per pos
    hib = np.concatenate([[0], np.cumsum(THI)])
    mb = np.concatenate([[0], np.cumsum(TLO + THI)])   # M chunk base per pos

    in_maps, orders = [], []
    for core in range(NCORES):
        br = branches[core // CORES_PER_BRANCH]
        pc = percore[core]
        idxlo = np.zeros(LOCH * P, np.int16)
        idxhi = np.zeros(HICH * P, np.int16)
        nedge = len(pc["s"])
        mrow = np.empty(nedge, np.int64)
        mcol = np.empty(nedge, np.int64)
        for pos in range(TILES):
            tl = int(pc["order"][pos])
            s0, s1 = int(pc["seg"][2 * tl]), int(pc["seg"][2 * tl + 1])
            k = np.arange(s1 - s0)
            idxlo[lob[pos] * P + k] = pc["s"][s0:s1]
            mrow[s0:s1] = k & 127
            mcol[s0:s1] = (mb[pos] + (k >> 7)) * P + pc["d128"][s0:s1]
            h0, h1 = s1, int(pc["seg"][2 * tl + 2])
            k = np.arange(h1 - h0)
            idxhi[hib[pos] * P + k] = pc["s"][h0:h1] - LO_LIMIT
            mrow[h0:h1] = k & 127
            mcol[h0:h1] = (mb[pos] + TLO[pos] + (k >> 7)) * P + pc["d128"][h0:h1]
        M = np.zeros((P, CHT * P), nbf)
        M[mrow, mcol] = pc["ww"]
        in_maps.append(dict(
            xt=br["xT"], xpt=br["xpT"], mpt=br["mpT"], mnt=br["mnT"],
            wmat=br["W"], bias=br["bias"],
            idxlo=_wrap_idx(idxlo), idxhi=_wrap_idx(idxhi), mmat=M))
        orders.append(pc["order"])

    return dict(TLO=TLO, THI=THI, in_maps=in_maps, orders=orders,
                b=[branches[0]["b"], branches[1]["b"]])


def _build(TLO, THI):
    LOCH, HICH = int(TLO.sum()), int(THI.sum())
    CHT = LOCH + HICH
    lob = np.concatenate([[0], np.cumsum(TLO)])
    hib = np.concatenate([[0], np.cumsum(THI)])
    mb = np.concatenate([[0], np.cumsum(TLO + THI)])

    nc = bacc.Bacc("TRN2", target_bir_lowering=False, debug=False,
                   num_devices=NCORES)
    xt_d = nc.dram_tensor("xt", [P, NN], bf16, kind="ExternalInput")
    xpt_d = nc.dram_tensor("xpt", [P, NN], bf16, kind="ExternalInput")
    mpt_d = nc.dram_tensor("mpt", [P, NN], bf16, kind="ExternalInput")
    mnt_d = nc.dram_tensor("mnt", [P, NN], bf16, kind="ExternalInput")
    w_d = nc.dram_tensor("wmat", [P, P], bf16, kind="ExternalInput")
    bias_d = nc.dram_tensor("bias", [P, FEAT], f32, kind="ExternalInput")
    idxlo_d = nc.dram_tensor("idxlo", [128, LOCH * 8], i16, kind="ExternalInput")
    idxhi_d = nc.dram_tensor("idxhi", [128, HICH * 8], i16, kind="ExternalInput")
    m_d = nc.dram_tensor("mmat", [P, CHT * P], bf16, kind="ExternalInput")
    ylo_d = nc.dram_tensor("ydat_lo", [LO_LIMIT, FEAT], bf16)
    yhi_d = nc.dram_tensor("ydat_hi", [NN - LO_LIMIT, FEAT], bf16)
    outp_d = nc.dram_tensor("outp", [TILES * P, FEAT], f32, kind="ExternalOutput")
    summ_d = nc.dram_tensor("summ", [1, P], f32, kind="ExternalOutput")

    with tile.TileContext(nc) as tc:
        with tc.tile_pool(name="const", bufs=1) as cp:
            w_t = cp.tile([P, P], bf16)
            nc.sync.dma_start(out=w_t[:], in_=w_d[:])
            bias_t = cp.tile([P, FEAT], f32)
            nc.sync.dma_start(out=bias_t[:], in_=bias_d[:])
            ones_t = cp.tile([P, 1], f32)
            nc.vector.memset(ones_t[:], 1.0)
            oacc = cp.tile([P, P], f32)
            nc.vector.memset(oacc[:], 0.0)
            idxlo_t = cp.tile([128, LOCH * 8], i16)
            nc.sync.dma_start(out=idxlo_t[:], in_=idxlo_d[:])
            idxhi_t = cp.tile([128, HICH * 8], i16)
            nc.sync.dma_start(out=idxhi_t[:], in_=idxhi_d[:])

            # ---- phase 1: y = [h_pos @ W | h_neg @ W], row-major bf16 ----
            with (
                tc.tile_pool(name="xw", bufs=2) as xp,
                tc.tile_pool(name="yps", bufs=3, space="PSUM") as yps,
                tc.tile_pool(name="ybp", bufs=2) as ybp,
            ):
                for s0 in range(0, NN, S_SUPER):
                    S = min(S_SUPER, NN - s0)
                    nchk = S // P
                    xtt = xp.tile([P, S], bf16, tag="xt")
                    nc.sync.dma_start(out=xtt[:], in_=xt_d[:, s0:s0 + S])
                    mptt = xp.tile([P, S], bf16, tag="mp")
                    nc.sync.dma_start(out=mptt[:], in_=mpt_d[:, s0:s0 + S])
                    hp = xp.tile([P, S], bf16, tag="hp")
                    nc.vector.tensor_mul(out=hp[:], in0=xtt[:], in1=mptt[:])
                    xptt = xp.tile([P, S], bf16, tag="xq")
                    nc.sync.dma_start(out=xptt[:], in_=xpt_d[:, s0:s0 + S])
                    mntt = xp.tile([P, S], bf16, tag="mn")
                    nc.sync.dma_start(out=mntt[:], in_=mnt_d[:, s0:s0 + S])
                    hn = xp.tile([P, S], bf16, tag="hn")
                    nc.vector.tensor_mul(out=hn[:], in0=xptt[:], in1=mntt[:])
                    yb = ybp.tile([P, nchk * FEAT], bf16, tag="yb")
                    for j in range(nchk):
                        ps = yps.tile([P, FEAT], f32, tag="ps")
                        nc.tensor.matmul(out=ps[:, 0:P],
                                         lhsT=hp[:, j * P:(j + 1) * P],
                                         rhs=w_t[:], start=True, stop=True)
                        nc.tensor.matmul(out=ps[:, P:FEAT],
                                         lhsT=hn[:, j * P:(j + 1) * P],
                                         rhs=w_t[:], start=True, stop=True)
                        if j % 2 == 0:
                            nc.scalar.copy(out=yb[:, j * FEAT:(j + 1) * FEAT],
                                           in_=ps[:])
                        else:
                            nc.vector.tensor_copy(out=yb[:, j * FEAT:(j + 1) * FEAT],
                                                  in_=ps[:])
                    if s0 < LO_LIMIT:
                        ydst = ylo_d[s0:s0 + S, :]
                    else:
                        ydst = yhi_d[s0 - LO_LIMIT:s0 - LO_LIMIT + S, :]
                    nc.sync.dma_start(
                        out=ydst.rearrange("(j p) f -> p j f", p=P),
                        in_=yb[:].rearrange("p (j f) -> p j f", f=FEAT))

            # ---- phase 2: gather + one-hot matmul aggregation ----
            with (
                tc.tile_pool(name="stream", bufs=2) as sp,
                tc.tile_pool(name="acc", bufs=2, space="PSUM") as ap2,
                tc.tile_pool(name="oep", bufs=3) as oep,
                tc.tile_pool(name="sps", bufs=1, space="PSUM") as spsp,
            ):
                for g0 in range(0, TILES, GROUP_TILES):
                    g1 = min(g0 + GROUP_TILES, TILES)
                    glo = int(TLO[g0:g1].sum())
                    ghi = int(THI[g0:g1].sum())
                    gch = glo + ghi
                    ylo = sp.tile([P, glo, FEAT], bf16, tag="ylo")
                    c = 0
                    while c < glo:
                        cn = min(MAX_GATHER_CH, glo - c)
                        ch0 = int(lob[g0]) + c
                        nc.gpsimd.dma_gather(
                            ylo[:, c:c + cn, :], ylo_d[:],
                            idxlo_t[:, ch0 * 8:(ch0 + cn) * 8],
                            cn * P, cn * P, FEAT, single_packet=False)
                        c += cn
                    yhi = sp.tile([P, ghi, FEAT], bf16, tag="yhi")
                    c = 0
                    while c < ghi:
                        cn = min(MAX_GATHER_CH, ghi - c)
                        ch0 = int(hib[g0]) + c
                        nc.gpsimd.dma_gather(
                            yhi[:, c:c + cn, :], yhi_d[:],
                            idxhi_t[:, ch0 * 8:(ch0 + cn) * 8],
                            cn * P, cn * P, FEAT, single_packet=False)
                        c += cn
                    mt = sp.tile([P, gch * P], bf16, tag="mt")
                    nc.sync.dma_start(
                        out=mt[:],
                        in_=m_d[:, int(mb[g0]) * P:int(mb[g0] + gch) * P])
                    lo_off = hi_off = m_off = 0
                    for pos in range(g0, g1):
                        ps = ap2.tile([P, FEAT], f32, tag="acc")
                        nchp = int(TLO[pos] + THI[pos])
                        k = 0
                        for cc in range(int(TLO[pos])):
                            nc.tensor.matmul(
                                out=ps[:],
                                lhsT=mt[:, (m_off + k) * P:(m_off + k + 1) * P],
                                rhs=ylo[:, lo_off + cc, :],
                                start=(k == 0), stop=(k == nchp - 1))
                            k += 1
                        for cc in range(int(THI[pos])):
                            nc.tensor.matmul(
                                out=ps[:],
                                lhsT=mt[:, (m_off + k) * P:(m_off + k + 1) * P],
                                rhs=yhi[:, hi_off + cc, :],
                                start=(k == 0), stop=(k == nchp - 1))
                            k += 1
                        ot = oep.tile([P, FEAT], f32, tag="ot")
                        nc.vector.tensor_add(out=ot[:], in0=ps[:], in1=bias_t[:])
                        nc.vector.tensor_scalar_max(out=ot[:], in0=ot[:],
                                                    scalar1=0.0)
                        nc.vector.tensor_add(out=oacc[:], in0=oacc[:],
                                             in1=ot[:, 0:P])
                        nc.sync.dma_start(out=outp_d[pos * P:(pos + 1) * P, :],
                                          in_=ot[:])
                        lo_off += int(TLO[pos])
                        hi_off += int(THI[pos])
                        m_off += nchp

                sps_t = spsp.tile([1, P], f32)
                nc.tensor.matmul(out=sps_t[:], lhsT=ones_t[:], rhs=oacc[:],
                                 start=True, stop=True)
                st = oep.tile([1, P], f32, tag="st")
                nc.vector.tensor_copy(out=st[:], in_=sps_t[:])
                nc.sync.dma_start(out=summ_d[:], in_=st[:])

    nc.compile()
    return nc


def _unshard(prep, results):
    outs = []
    for bi in range(2):
        p_full = np.empty((N, P), np.float32)
        n_full = np.empty((N, P), np.float32)
        s_acc = np.zeros(P, np.float64)
        relu_b = np.maximum(prep["b"][bi], 0.0)
        for ci in range(CORES_PER_BRANCH):
            core = bi * CORES_PER_BRANCH + ci
            base = ci * DST_PER_CORE
            o = results[core]["outp"].reshape(TILES, P, FEAT)
            order = prep["orders"][core]
            for pos in range(TILES):
                t = int(order[pos])
                r0 = base + t * P
                nrows = min(P, base + DST_PER_CORE - r0)
                p_full[r0:r0 + nrows] = o[pos, :nrows, 0:P]
                n_full[r0:r0 + nrows] = o[pos, :nrows, P:FEAT]
            s_acc += results[core]["summ"][0].astype(np.float64) \
                - PAD_ROWS * relu_b
        s = (s_acc / N).astype(np.float32).reshape(1, P)
        outs.extend([p_full, n_full, s])
    return (outs[0], outs[1], outs[2], outs[3], outs[4], outs[5])


_RUN_KWARGS = {}


def kernel(**inputs):
    prep = _prep(inputs)
    nc = _build(prep["TLO"], prep["THI"])
    res = run_bass_kernel_spmd(nc, prep["in_maps"], list(range(NCORES)),
                               **_RUN_KWARGS)
    out = _unshard(prep, res.results)
    kernel.last_result = res
    return out


# revision 5
# speedup vs baseline: 1.1881x; 1.1881x over previous
"""Trainium2 Bass kernel: two-branch GCN embedding (DMGI-style).

Computation per branch (see reference):
    pos_h = relu(gcn_conv(x * mask_pos, W, b, edge_index))
    neg_h = relu(gcn_conv((x * mask_neg)[perm], W, b, edge_index))
    summary = mean(pos_h, axis=0)

Strategy (8 NeuronCores, SPMD, no collectives):
  - Cores 0-3 run branch 1, cores 4-7 branch 2; each core owns a quarter of
    the destination nodes of its branch.
  - Host prep: transpose/cast inputs to bf16 (feature-major), absorb the neg
    permutation into a permuted copy of x / mask_neg, bucket edges
    (+self-loops) by destination tile, compute symmetric norms, and build
    per-128-edge-chunk one-hot matrices M (M[e, d] = norm_e iff dst_e == d).
  - Device phase 1: y = [(x.mask_pos) @ W | (x_perm.mask_neg_perm) @ W] for
    all 50k nodes (replicated per core), stored row-major bf16 in DRAM.
  - Device phase 2: for each destination tile, dma_gather the y rows of its
    incoming edges (512B rows) and accumulate PSUM[dst,feat] via
    out += M_chunk^T @ Y_chunk on the TensorEngine; epilogue adds bias,
    applies relu, stores, and accumulates the summary.
  - Host unshard: reassemble rows, sum partial summaries.

dma_gather indices are int16, so each tile's edges are split into
src < 32768 ("lo") and src >= 32768 ("hi", gathered from a base-offset view).
Per-position chunk counts are data-dependent but must be uniform across cores
(SPMD shares one NEFF): counts are maxed across cores after sorting each
core's tiles by edge count (bin-packing keeps the padding waste ~3%).
"""

import numpy as np
import ml_dtypes

import concourse.bacc as bacc
import concourse.mybir as mybir
import concourse.tile as tile
from concourse.bass_utils import run_bass_kernel_spmd

# ---- hardcoded problem constants ----
N = 50000
D = 128
P = 128
FEAT = 256                       # fused pos|neg row width
NN = 50048                       # N padded to 391*128
NCORES = 8
CORES_PER_BRANCH = 4
DST_PER_CORE = N // CORES_PER_BRANCH      # 12500
TILES = (DST_PER_CORE + P - 1) // P       # 98
PAD_ROWS = TILES * P - DST_PER_CORE       # 44
LO_LIMIT = 32768
S_SUPER = 4096
GROUP_TILES = 4
MAX_GATHER_CH = 32               # chunks (x128 idx) per dma_gather call

bf16 = mybir.dt.bfloat16
f32 = mybir.dt.float32
i16 = mybir.dt.int16
nbf = ml_dtypes.bfloat16


def _wrap_idx(a):
    """[n] int16 -> [128, n//16] wrapped (j at [j%16, j//16]) replicated x8."""
    return np.tile(a.reshape(-1, 16).T, (8, 1)).copy()


def _prep(inputs):
    x = np.asarray(inputs["x"], np.float32)
    branches = []
    for bi in (1, 2):
        W = np.asarray(inputs[f"W{bi}"], np.float32)
        b = np.asarray(inputs[f"b{bi}"], np.float32)
        mp = np.asarray(inputs[f"mask_pos{bi}"], np.float32)
        mn = np.asarray(inputs[f"mask_neg{bi}"], np.float32)
        ei = np.asarray(inputs[f"edge_index{bi}"])
        pm = np.asarray(inputs[f"perm{bi}"]).astype(np.int64)
        src = np.concatenate([ei[0].astype(np.int64), np.arange(N, dtype=np.int64)])
        dst = np.concatenate([ei[1].astype(np.int64), np.arange(N, dtype=np.int64)])
        deg = np.bincount(dst, minlength=N).astype(np.float64)
        dinv = 1.0 / np.sqrt(np.maximum(deg, 1.0))
        w = (dinv[src] * dinv[dst]).astype(np.float32)

        xT = np.zeros((P, NN), nbf)
        xT[:, :N] = x.T
        xpT = np.zeros((P, NN), nbf)
        xpT[:, :N] = x[pm].T
        mpT = np.zeros((P, NN), nbf)
        mpT[:, :N] = mp.T
        mnT = np.zeros((P, NN), nbf)
        mnT[:, :N] = mn[pm].T
        bias = np.broadcast_to(
            np.concatenate([b, b]).astype(nbf), (P, FEAT)).copy()
        branches.append(dict(
            W=np.ascontiguousarray(W.astype(nbf)), b=b, bias=bias,
            xT=xT, xpT=xpT, mpT=mpT, mnT=mnT, src=src, dst=dst, w=w))

    # per-core edge bucketing
    percore = []
    for core in range(NCORES):
        br = branches[core // CORES_PER_BRANCH]
        base = (core % CORES_PER_BRANCH) * DST_PER_CORE
        sel = (br["dst"] >= base) & (br["dst"] < base + DST_PER_CORE)
        s = br["src"][sel]
        dl = br["dst"][sel] - base
        ww = br["w"][sel]
        t = dl >> 7
        d128 = dl & 127
        hi = (s >= LO_LIMIT).astype(np.int64)
        key = t * 2 + hi
        o = np.argsort(key, kind="stable")
        s, d128, ww, key = s[o], d128[o], ww[o], key[o]
        cnt = np.bincount(key, minlength=TILES * 2)
        seg = np.concatenate([[0], np.cumsum(cnt)])
        locnt, hicnt = cnt[0::2], cnt[1::2]
        order = np.argsort(-locnt, kind="stable")      # position -> tile id
        percore.append(dict(s=s, d128=d128, ww=ww, seg=seg,
                            locnt=locnt, hicnt=hicnt, order=order))

    # global per-position chunk counts (uniform across cores)
    TLO = np.zeros(TILES, np.int64)
    THI = np.zeros(TILES, np.int64)
    for pc in percore:
        TLO = np.maximum(TLO, -(-pc["locnt"][pc["order"]] // P))
        THI = np.maximum(THI, -(-pc["hicnt"][pc["order"]] // P))
    TLO = TLO.astype(np.int64)
    THI = THI.astype(np.int64)
    LOCH, HICH = int(TLO.sum()), int(THI.sum())
    CHT = LOCH + HICH
    lob = np.concatenate([[0], np.cumsum(TLO)])        # lo chunk base per pos
    hib = np.concatenate([[0], np.cumsum(THI)])
    mb = np.concatenate([[0], np.cumsum(TLO + THI)])   # M chunk base per pos

    in_maps, orders = [], []
    for core in range(NCORES):
        br = branches[core // CORES_PER_BRANCH]
        pc = percore[core]
        idxlo = np.zeros(LOCH * P, np.int16)
        idxhi = np.zeros(HICH * P, np.int16)
        nedge = len(pc["s"])
        mrow = np.empty(nedge, np.int64)
        mcol = np.empty(nedge, np.int64)
        for pos in range(TILES):
            tl = int(pc["order"][pos])
            s0, s1 = int(pc["seg"][2 * tl]), int(pc["seg"][2 * tl + 1])
            k = np.arange(s1 - s0)
            idxlo[lob[pos] * P + k] = pc["s"][s0:s1]
            mrow[s0:s1] = k & 127
            mcol[s0:s1] = (mb[pos] + (k >> 7)) * P + pc["d128"][s0:s1]
            h0, h1 = s1, int(pc["seg"][2 * tl + 2])
            k = np.arange(h1 - h0)
            idxhi[hib[pos] * P + k] = pc["s"][h0:h1] - LO_LIMIT
            mrow[h0:h1] = k & 127
            mcol[h0:h1] = (mb[pos] + TLO[pos] + (k >> 7)) * P + pc["d128"][h0:h1]
        M = np.zeros((P, CHT * P), nbf)
        M[mrow, mcol] = pc["ww"]
        in_maps.append(dict(
            xt=br["xT"], xpt=br["xpT"], mpt=br["mpT"], mnt=br["mnT"],
            wmat=br["W"], bias=br["bias"],
            idxlo=_wrap_idx(idxlo), idxhi=_wrap_idx(idxhi), mmat=M))
        orders.append(pc["order"])

    return dict(TLO=TLO, THI=THI, in_maps=in_maps, orders=orders,
                b=[branches[0]["b"], branches[1]["b"]])


def _build(TLO, THI):
    LOCH, HICH = int(TLO.sum()), int(THI.sum())
    CHT = LOCH + HICH
    lob = np.concatenate([[0], np.cumsum(TLO)])
    hib = np.concatenate([[0], np.cumsum(THI)])
    mb = np.concatenate([[0], np.cumsum(TLO + THI)])

    nc = bacc.Bacc("TRN2", target_bir_lowering=False, debug=False,
                   num_devices=NCORES)
    xt_d = nc.dram_tensor("xt", [P, NN], bf16, kind="ExternalInput")
    xpt_d = nc.dram_tensor("xpt", [P, NN], bf16, kind="ExternalInput")
    mpt_d = nc.dram_tensor("mpt", [P, NN], bf16, kind="ExternalInput")
    mnt_d = nc.dram_tensor("mnt", [P, NN], bf16, kind="ExternalInput")
    w_d = nc.dram_tensor("wmat", [P, P], bf16, kind="ExternalInput")
    bias_d = nc.dram_tensor("bias", [P, FEAT], bf16, kind="ExternalInput")
    idxlo_d = nc.dram_tensor("idxlo", [128, LOCH * 8], i16, kind="ExternalInput")
    idxhi_d = nc.dram_tensor("idxhi", [128, HICH * 8], i16, kind="ExternalInput")
    m_d = nc.dram_tensor("mmat", [P, CHT * P], bf16, kind="ExternalInput")
    ylo_d = nc.dram_tensor("ydat_lo", [LO_LIMIT, FEAT], bf16)
    yhi_d = nc.dram_tensor("ydat_hi", [NN - LO_LIMIT, FEAT], bf16)
    outp_d = nc.dram_tensor("outp", [TILES * P, FEAT], f32, kind="ExternalOutput")
    summ_d = nc.dram_tensor("summ", [1, P], f32, kind="ExternalOutput")

    with tile.TileContext(nc) as tc:
        with tc.tile_pool(name="const", bufs=1) as cp:
            w_t = cp.tile([P, P], bf16)
            nc.sync.dma_start(out=w_t[:], in_=w_d[:])
            bias_t = cp.tile([P, FEAT], bf16)
            nc.sync.dma_start(out=bias_t[:], in_=bias_d[:])
            mbias_t = cp.tile([P, P], bf16)
            nc.vector.memset(mbias_t[:], 1.0 / 128.0)
            ones_t = cp.tile([P, 1], f32)
            nc.vector.memset(ones_t[:], 1.0)
            oacc = cp.tile([P, P], f32)
            nc.vector.memset(oacc[:], 0.0)
            idxlo_t = cp.tile([128, LOCH * 8], i16)
            nc.sync.dma_start(out=idxlo_t[:], in_=idxlo_d[:])
            idxhi_t = cp.tile([128, HICH * 8], i16)
            nc.sync.dma_start(out=idxhi_t[:], in_=idxhi_d[:])

            # ---- phase 1: y = [h_pos @ W | h_neg @ W], row-major bf16 ----
            with (
                tc.tile_pool(name="xw", bufs=2) as xp,
                tc.tile_pool(name="yps", bufs=3, space="PSUM") as yps,
                tc.tile_pool(name="ybp", bufs=2) as ybp,
            ):
                for s0 in range(0, NN, S_SUPER):
                    S = min(S_SUPER, NN - s0)
                    nchk = S // P
                    xtt = xp.tile([P, S], bf16, tag="xt")
                    nc.sync.dma_start(out=xtt[:], in_=xt_d[:, s0:s0 + S])
                    mptt = xp.tile([P, S], bf16, tag="mp")
                    nc.sync.dma_start(out=mptt[:], in_=mpt_d[:, s0:s0 + S])
                    hp = xp.tile([P, S], bf16, tag="hp")
                    nc.vector.tensor_mul(out=hp[:], in0=xtt[:], in1=mptt[:])
                    xptt = xp.tile([P, S], bf16, tag="xq")
                    nc.sync.dma_start(out=xptt[:], in_=xpt_d[:, s0:s0 + S])
                    mntt = xp.tile([P, S], bf16, tag="mn")
                    nc.sync.dma_start(out=mntt[:], in_=mnt_d[:, s0:s0 + S])
                    hn = xp.tile([P, S], bf16, tag="hn")
                    nc.vector.tensor_mul(out=hn[:], in0=xptt[:], in1=mntt[:])
                    yb = ybp.tile([P, nchk * FEAT], bf16, tag="yb")
                    for j in range(nchk):
                        ps = yps.tile([P, FEAT], f32, tag="ps")
                        nc.tensor.matmul(out=ps[:, 0:P],
                                         lhsT=hp[:, j * P:(j + 1) * P],
                                         rhs=w_t[:], start=True, stop=True)
                        nc.tensor.matmul(out=ps[:, P:FEAT],
                                         lhsT=hn[:, j * P:(j + 1) * P],
                                         rhs=w_t[:], start=True, stop=True)
                        nc.scalar.copy(out=yb[:, j * FEAT:(j + 1) * FEAT],
                                       in_=ps[:])
                    if s0 < LO_LIMIT:
                        ydst = ylo_d[s0:s0 + S, :]
                    else:
                        ydst = yhi_d[s0 - LO_LIMIT:s0 - LO_LIMIT + S, :]
                    nc.sync.dma_start(
                        out=ydst.rearrange("(j p) f -> p j f", p=P),
                        in_=yb[:].rearrange("p (j f) -> p j f", f=FEAT))

            # ---- phase 2: gather + one-hot matmul aggregation ----
            with (
                tc.tile_pool(name="stream", bufs=2) as sp,
                tc.tile_pool(name="acc", bufs=3, space="PSUM") as ap2,
                tc.tile_pool(name="oep", bufs=3) as oep,
                tc.tile_pool(name="sps", bufs=1, space="PSUM") as spsp,
            ):
                for g0 in range(0, TILES, GROUP_TILES):
                    g1 = min(g0 + GROUP_TILES, TILES)
                    glo = int(TLO[g0:g1].sum())
                    ghi = int(THI[g0:g1].sum())
                    gch = glo + ghi
                    ylo = sp.tile([P, glo, FEAT], bf16, tag="ylo")
                    c = 0
                    while c < glo:
                        cn = min(MAX_GATHER_CH, glo - c)
                        ch0 = int(lob[g0]) + c
                        nc.gpsimd.dma_gather(
                            ylo[:, c:c + cn, :], ylo_d[:],
                            idxlo_t[:, ch0 * 8:(ch0 + cn) * 8],
                            cn * P, cn * P, FEAT, single_packet=False)
                        c += cn
                    yhi = sp.tile([P, ghi, FEAT], bf16, tag="yhi")
                    c = 0
                    while c < ghi:
                        cn = min(MAX_GATHER_CH, ghi - c)
                        ch0 = int(hib[g0]) + c
                        nc.gpsimd.dma_gather(
                            yhi[:, c:c + cn, :], yhi_d[:],
                            idxhi_t[:, ch0 * 8:(ch0 + cn) * 8],
                            cn * P, cn * P, FEAT, single_packet=False)
                        c += cn
                    mt = sp.tile([P, gch * P], bf16, tag="mt")
                    nc.sync.dma_start(
                        out=mt[:],
                        in_=m_d[:, int(mb[g0]) * P:int(mb[g0] + gch) * P])
                    lo_off = hi_off = m_off = 0
                    for pos in range(g0, g1):
                        ps = ap2.tile([P, FEAT], f32, tag="acc")
                        nchp = int(TLO[pos] + THI[pos])
                        k = 0
                        for cc in range(int(TLO[pos])):
                            nc.tensor.matmul(
                                out=ps[:],
                                lhsT=mt[:, (m_off + k) * P:(m_off + k + 1) * P],
                                rhs=ylo[:, lo_off + cc, :],
                                start=(k == 0), stop=False)
                            k += 1
                        for cc in range(int(THI[pos])):
                            nc.tensor.matmul(
                                out=ps[:],
                                lhsT=mt[:, (m_off + k) * P:(m_off + k + 1) * P],
                                rhs=yhi[:, hi_off + cc, :],
                                start=(k == 0), stop=False)
                            k += 1
                        nc.tensor.matmul(
                            out=ps[:], lhsT=mbias_t[:], rhs=bias_t[:],
                            start=(k == 0), stop=True)
                        ot = oep.tile([P, FEAT], f32, tag="ot")
                        nc.scalar.activation(
                            out=ot[:], in_=ps[:],
                            func=mybir.ActivationFunctionType.Relu)
                        nc.vector.tensor_add(out=oacc[:], in0=oacc[:],
                                             in1=ot[:, 0:P])
                        nc.sync.dma_start(out=outp_d[pos * P:(pos + 1) * P, :],
                                          in_=ot[:])
                        lo_off += int(TLO[pos])
                        hi_off += int(THI[pos])
                        m_off += nchp

                sps_t = spsp.tile([1, P], f32)
                nc.tensor.matmul(out=sps_t[:], lhsT=ones_t[:], rhs=oacc[:],
                                 start=True, stop=True)
                st = oep.tile([1, P], f32, tag="st")
                nc.vector.tensor_copy(out=st[:], in_=sps_t[:])
                nc.sync.dma_start(out=summ_d[:], in_=st[:])

    nc.compile()
    return nc


def _unshard(prep, results):
    outs = []
    for bi in range(2):
        p_full = np.empty((N, P), np.float32)
        n_full = np.empty((N, P), np.float32)
        s_acc = np.zeros(P, np.float64)
        relu_b = np.maximum(prep["b"][bi], 0.0)
        for ci in range(CORES_PER_BRANCH):
            core = bi * CORES_PER_BRANCH + ci
            base = ci * DST_PER_CORE
            o = results[core]["outp"].reshape(TILES, P, FEAT)
            order = prep["orders"][core]
            for pos in range(TILES):
                t = int(order[pos])
                r0 = base + t * P
                nrows = min(P, base + DST_PER_CORE - r0)
                p_full[r0:r0 + nrows] = o[pos, :nrows, 0:P]
                n_full[r0:r0 + nrows] = o[pos, :nrows, P:FEAT]
            s_acc += results[core]["summ"][0].astype(np.float64) \
                - PAD_ROWS * relu_b
        s = (s_acc / N).astype(np.float32).reshape(1, P)
        outs.extend([p_full, n_full, s])
    return (outs[0], outs[1], outs[2], outs[3], outs[4], outs[5])


_RUN_KWARGS = {}


def kernel(**inputs):
    prep = _prep(inputs)
    nc = _build(prep["TLO"], prep["THI"])
    res = run_bass_kernel_spmd(nc, prep["in_maps"], list(range(NCORES)),
                               **_RUN_KWARGS)
    out = _unshard(prep, res.results)
    kernel.last_result = res
    return out


# revision 8
# speedup vs baseline: 1.9854x; 1.6710x over previous
"""Trainium2 Bass kernel: two-branch GCN embedding (DMGI-style).

Computation per branch (see reference):
    pos_h = relu(gcn_conv(x * mask_pos, W, b, edge_index))
    neg_h = relu(gcn_conv((x * mask_neg)[perm], W, b, edge_index))
    summary = mean(pos_h, axis=0)

Strategy (8 NeuronCores, SPMD, no collectives):
  - Cores 0-3 run branch 1, cores 4-7 branch 2; each core owns a quarter of
    the destination nodes of its branch.
  - Host prep: transpose/cast inputs to bf16 (feature-major), absorb the neg
    permutation into a permuted copy of x / mask_neg, bucket edges
    (+self-loops) by destination tile, compute symmetric norms, and build
    per-128-edge-chunk one-hot matrices M (M[e, d] = norm_e iff dst_e == d).
  - Device phase 1: y = [(x.mask_pos) @ W | (x_perm.mask_neg_perm) @ W] for
    all 50k nodes (replicated per core), stored row-major bf16 in DRAM.
  - Device phase 2: for each destination tile, dma_gather the y rows of its
    incoming edges (512B rows) and accumulate PSUM[dst,feat] via
    out += M_chunk^T @ Y_chunk on the TensorEngine; epilogue adds bias,
    applies relu, stores, and accumulates the summary.
  - Host unshard: reassemble rows, sum partial summaries.

dma_gather indices are int16, so each tile's edges are split into
src < 32768 ("lo") and src >= 32768 ("hi", gathered from a base-offset view).
Per-position chunk counts are data-dependent but must be uniform across cores
(SPMD shares one NEFF): counts are maxed across cores after sorting each
core's tiles by edge count (bin-packing keeps the padding waste ~3%).
"""

import numpy as np
import ml_dtypes

import concourse.bacc as bacc
import concourse.mybir as mybir
import concourse.tile as tile
from concourse.bass_utils import run_bass_kernel_spmd

# ---- hardcoded problem constants ----
N = 50000
D = 128
P = 128
FEAT = 256                       # fused pos|neg row width
NN = 50048                       # N padded to 391*128
NCORES = 8
CORES_PER_BRANCH = 4
DST_PER_CORE = N // CORES_PER_BRANCH      # 12500
TILES = (DST_PER_CORE + P - 1) // P       # 98
PAD_ROWS = TILES * P - DST_PER_CORE       # 44
LO_LIMIT = 32768
S_SUPER = 4096
GROUP_TILES = 4
MAX_GATHER_CH = 32               # chunks (x128 idx) per dma_gather call

bf16 = mybir.dt.bfloat16
f32 = mybir.dt.float32
i16 = mybir.dt.int16
nbf = ml_dtypes.bfloat16


def _wrap_idx(a):
    """[n] int16 -> [128, n//16] wrapped (j at [j%16, j//16]) replicated x8."""
    return np.tile(a.reshape(-1, 16).T, (8, 1)).copy()


def _prep(inputs):
    x = np.asarray(inputs["x"], np.float32)
    branches = []
    for bi in (1, 2):
        W = np.asarray(inputs[f"W{bi}"], np.float32)
        b = np.asarray(inputs[f"b{bi}"], np.float32)
        mp = np.asarray(inputs[f"mask_pos{bi}"], np.float32)
        mn = np.asarray(inputs[f"mask_neg{bi}"], np.float32)
        ei = np.asarray(inputs[f"edge_index{bi}"])
        pm = np.asarray(inputs[f"perm{bi}"]).astype(np.int64)
        src = np.concatenate([ei[0].astype(np.int64), np.arange(N, dtype=np.int64)])
        dst = np.concatenate([ei[1].astype(np.int64), np.arange(N, dtype=np.int64)])
        deg = np.bincount(dst, minlength=N).astype(np.float64)
        dinv = 1.0 / np.sqrt(np.maximum(deg, 1.0))
        w = (dinv[src] * dinv[dst]).astype(np.float32)

        xT = np.zeros((P, NN), nbf)
        xT[:, :N] = x.T
        xpT = np.zeros((P, NN), nbf)
        xpT[:, :N] = x[pm].T
        mpT = np.zeros((P, NN), nbf)
        mpT[:, :N] = mp.T
        mnT = np.zeros((P, NN), nbf)
        mnT[:, :N] = mn[pm].T
        bias = np.broadcast_to(
            np.concatenate([b, b]).astype(nbf), (P, FEAT)).copy()
        branches.append(dict(
            W=np.ascontiguousarray(W.astype(nbf)), b=b, bias=bias,
            xT=xT, xpT=xpT, mpT=mpT, mnT=mnT, src=src, dst=dst, w=w))

    # snake-deal global dst tiles to the branch's 4 cores by lo-edge count:
    # per-position counts align across cores, minimizing cross-core pad slack
    GT = (N + P - 1) // P                     # 391 global dst tiles
    for br in branches:
        t_all = (br["dst"] >> 7).astype(np.int64)
        locnt_g = np.bincount(t_all[br["src"] < LO_LIMIT], minlength=GT)
        order_g = np.argsort(-locnt_g, kind="stable")
        coreof = np.empty(GT, np.int64)
        posof = np.empty(GT, np.int64)
        corelist = [[] for _ in range(CORES_PER_BRANCH)]
        for r, tid in enumerate(order_g):
            k = r % (2 * CORES_PER_BRANCH)
            c = k if k < CORES_PER_BRANCH else 2 * CORES_PER_BRANCH - 1 - k
            coreof[tid] = c
            posof[tid] = len(corelist[c])
            corelist[c].append(int(tid))
        br["coreof"], br["posof"], br["corelist"] = coreof, posof, corelist

    # per-core edge bucketing by (position, lo/hi)
    percore = []
    for core in range(NCORES):
        br = branches[core // CORES_PER_BRANCH]
        ci = core % CORES_PER_BRANCH
        sel = br["coreof"][(br["dst"] >> 7)] == ci
        s = br["src"][sel]
        d = br["dst"][sel]
        ww = br["w"][sel]
        pos_e = br["posof"][d >> 7]
        d128 = d & 127
        hi = (s >= LO_LIMIT).astype(np.int64)
        key = pos_e * 2 + hi
        o = np.argsort(key, kind="stable")
        s, d128, ww = s[o], d128[o], ww[o]
        cnt = np.bincount(key[o], minlength=TILES * 2)
        seg = np.concatenate([[0], np.cumsum(cnt)])
        locnt, hicnt = cnt[0::2], cnt[1::2]
        percore.append(dict(s=s, d128=d128, ww=ww, seg=seg,
                            locnt=locnt, hicnt=hicnt,
                            corelist=br["corelist"][ci]))

    # global per-position chunk counts (uniform across cores)
    TLO = np.zeros(TILES, np.int64)
    THI = np.zeros(TILES, np.int64)
    for pc in percore:
        TLO = np.maximum(TLO, -(-pc["locnt"] // P))
        THI = np.maximum(THI, -(-pc["hicnt"] // P))
    TLO = TLO.astype(np.int64)
    THI = THI.astype(np.int64)
    LOCH, HICH = int(TLO.sum()), int(THI.sum())
    CHT = LOCH + HICH
    lob = np.concatenate([[0], np.cumsum(TLO)])        # lo chunk base per pos
    hib = np.concatenate([[0], np.cumsum(THI)])
    mb = np.concatenate([[0], np.cumsum(TLO + THI)])   # M chunk base per pos

    in_maps, orders = [], []
    for core in range(NCORES):
        br = branches[core // CORES_PER_BRANCH]
        pc = percore[core]
        idxlo = np.zeros(LOCH * P, np.int16)
        idxhi = np.zeros(HICH * P, np.int16)
        nedge = len(pc["s"])
        mrow = np.empty(nedge, np.int64)
        mcol = np.empty(nedge, np.int64)
        for pos in range(TILES):
            s0, s1 = int(pc["seg"][2 * pos]), int(pc["seg"][2 * pos + 1])
            k = np.arange(s1 - s0)
            idxlo[lob[pos] * P + k] = pc["s"][s0:s1]
            mrow[s0:s1] = k & 127
            mcol[s0:s1] = (mb[pos] + (k >> 7)) * P + pc["d128"][s0:s1]
            h0, h1 = s1, int(pc["seg"][2 * pos + 2])
            k = np.arange(h1 - h0)
            idxhi[hib[pos] * P + k] = pc["s"][h0:h1] - LO_LIMIT
            mrow[h0:h1] = k & 127
            mcol[h0:h1] = (mb[pos] + TLO[pos] + (k >> 7)) * P + pc["d128"][h0:h1]
        M = np.zeros((P, CHT * P), nbf)
        M[mrow, mcol] = pc["ww"]
        in_maps.append(dict(
            xt=br["xT"], xpt=br["xpT"], mpt=br["mpT"], mnt=br["mnT"],
            wmat=br["W"], bias=br["bias"],
            idxlo=_wrap_idx(idxlo), idxhi=_wrap_idx(idxhi), mmat=M))
        orders.append(pc["corelist"])

    return dict(TLO=TLO, THI=THI, in_maps=in_maps, orders=orders,
                b=[branches[0]["b"], branches[1]["b"]])


def _build(TLO, THI):
    LOCH, HICH = int(TLO.sum()), int(THI.sum())
    CHT = LOCH + HICH
    lob = np.concatenate([[0], np.cumsum(TLO)])
    hib = np.concatenate([[0], np.cumsum(THI)])
    mb = np.concatenate([[0], np.cumsum(TLO + THI)])

    nc = bacc.Bacc("TRN2", target_bir_lowering=False, debug=False,
                   num_devices=NCORES, num_swdge_queues=2)
    xt_d = nc.dram_tensor("xt", [P, NN], bf16, kind="ExternalInput")
    xpt_d = nc.dram_tensor("xpt", [P, NN], bf16, kind="ExternalInput")
    mpt_d = nc.dram_tensor("mpt", [P, NN], bf16, kind="ExternalInput")
    mnt_d = nc.dram_tensor("mnt", [P, NN], bf16, kind="ExternalInput")
    w_d = nc.dram_tensor("wmat", [P, P], bf16, kind="ExternalInput")
    bias_d = nc.dram_tensor("bias", [P, FEAT], bf16, kind="ExternalInput")
    idxlo_d = nc.dram_tensor("idxlo", [128, LOCH * 8], i16, kind="ExternalInput")
    idxhi_d = nc.dram_tensor("idxhi", [128, HICH * 8], i16, kind="ExternalInput")
    m_d = nc.dram_tensor("mmat", [P, CHT * P], bf16, kind="ExternalInput")
    ylo_d = nc.dram_tensor("ydat_lo", [LO_LIMIT, FEAT], bf16)
    yhi_d = nc.dram_tensor("ydat_hi", [NN - LO_LIMIT, FEAT], bf16)
    outp_d = nc.dram_tensor("outp", [TILES * P, FEAT], f32, kind="ExternalOutput")
    summ_d = nc.dram_tensor("summ", [1, P], f32, kind="ExternalOutput")

    with tile.TileContext(nc) as tc:
        with tc.tile_pool(name="const", bufs=1) as cp:
            w_t = cp.tile([P, P], bf16)
            nc.sync.dma_start(out=w_t[:], in_=w_d[:])
            bias_t = cp.tile([P, FEAT], bf16)
            nc.sync.dma_start(out=bias_t[:], in_=bias_d[:])
            mbias_t = cp.tile([P, P], bf16)
            nc.vector.memset(mbias_t[:], 1.0 / 128.0)
            ones_t = cp.tile([P, 1], f32)
            nc.vector.memset(ones_t[:], 1.0)
            oacc = cp.tile([P, P], f32)
            nc.vector.memset(oacc[:], 0.0)
            idxlo_t = cp.tile([128, LOCH * 8], i16)
            nc.sync.dma_start(out=idxlo_t[:], in_=idxlo_d[:])
            idxhi_t = cp.tile([128, HICH * 8], i16)
            nc.sync.dma_start(out=idxhi_t[:], in_=idxhi_d[:])

            # ---- phase 1: y = [h_pos @ W | h_neg @ W], row-major bf16 ----
            with (
                tc.tile_pool(name="xw", bufs=2) as xp,
                tc.tile_pool(name="yps", bufs=3, space="PSUM") as yps,
                tc.tile_pool(name="ybp", bufs=2) as ybp,
            ):
                for s0 in range(0, NN, S_SUPER):
                    S = min(S_SUPER, NN - s0)
                    nchk = S // P
                    xtt = xp.tile([P, S], bf16, tag="xt")
                    nc.sync.dma_start(out=xtt[:], in_=xt_d[:, s0:s0 + S])
                    mptt = xp.tile([P, S], bf16, tag="mp")
                    nc.sync.dma_start(out=mptt[:], in_=mpt_d[:, s0:s0 + S])
                    hp = xp.tile([P, S], bf16, tag="hp")
                    nc.vector.tensor_mul(out=hp[:], in0=xtt[:], in1=mptt[:])
                    xptt = xp.tile([P, S], bf16, tag="xq")
                    nc.sync.dma_start(out=xptt[:], in_=xpt_d[:, s0:s0 + S])
                    mntt = xp.tile([P, S], bf16, tag="mn")
                    nc.sync.dma_start(out=mntt[:], in_=mnt_d[:, s0:s0 + S])
                    hn = xp.tile([P, S], bf16, tag="hn")
                    nc.vector.tensor_mul(out=hn[:], in0=xptt[:], in1=mntt[:])
                    yb = ybp.tile([P, nchk * FEAT], bf16, tag="yb")
                    for j in range(nchk):
                        ps = yps.tile([P, FEAT], f32, tag="ps")
                        nc.tensor.matmul(out=ps[:, 0:P],
                                         lhsT=hp[:, j * P:(j + 1) * P],
                                         rhs=w_t[:], start=True, stop=True)
                        nc.tensor.matmul(out=ps[:, P:FEAT],
                                         lhsT=hn[:, j * P:(j + 1) * P],
                                         rhs=w_t[:], start=True, stop=True)
                        nc.scalar.copy(out=yb[:, j * FEAT:(j + 1) * FEAT],
                                       in_=ps[:])
                    if s0 < LO_LIMIT:
                        ydst = ylo_d[s0:s0 + S, :]
                    else:
                        ydst = yhi_d[s0 - LO_LIMIT:s0 - LO_LIMIT + S, :]
                    nc.sync.dma_start(
                        out=ydst.rearrange("(j p) f -> p j f", p=P),
                        in_=yb[:].rearrange("p (j f) -> p j f", f=FEAT))

            # ---- phase 2: gather + one-hot matmul aggregation ----
            with (
                tc.tile_pool(name="stream", bufs=2) as sp,
                tc.tile_pool(name="acc", bufs=3, space="PSUM") as ap2,
                tc.tile_pool(name="oep", bufs=3) as oep,
                tc.tile_pool(name="sps", bufs=1, space="PSUM") as spsp,
            ):
                qrr = 0
                for g0 in range(0, TILES, GROUP_TILES):
                    g1 = min(g0 + GROUP_TILES, TILES)
                    glo = int(TLO[g0:g1].sum())
                    ghi = int(THI[g0:g1].sum())
                    gch = glo + ghi
                    ylo = sp.tile([P, glo, FEAT], bf16, tag="ylo")
                    c = 0
                    while c < glo:
                        cn = min(MAX_GATHER_CH, glo - c)
                        ch0 = int(lob[g0]) + c
                        nc.gpsimd.dma_gather(
                            ylo[:, c:c + cn, :], ylo_d[:],
                            idxlo_t[:, ch0 * 8:(ch0 + cn) * 8],
                            cn * P, cn * P, FEAT, single_packet=False,
                            queue_num=qrr % 2)
                        qrr += 1
                        c += cn
                    yhi = sp.tile([P, ghi, FEAT], bf16, tag="yhi")
                    c = 0
                    while c < ghi:
                        cn = min(MAX_GATHER_CH, ghi - c)
                        ch0 = int(hib[g0]) + c
                        nc.gpsimd.dma_gather(
                            yhi[:, c:c + cn, :], yhi_d[:],
                            idxhi_t[:, ch0 * 8:(ch0 + cn) * 8],
                            cn * P, cn * P, FEAT, single_packet=False,
                            queue_num=qrr % 2)
                        qrr += 1
                        c += cn
                    mt = sp.tile([P, gch * P], bf16, tag="mt")
                    nc.sync.dma_start(
                        out=mt[:],
                        in_=m_d[:, int(mb[g0]) * P:int(mb[g0] + gch) * P])
                    lo_off = hi_off = m_off = 0
                    for pos in range(g0, g1):
                        ps = ap2.tile([P, FEAT], f32, tag="acc")
                        nchp = int(TLO[pos] + THI[pos])
                        k = 0
                        for cc in range(int(TLO[pos])):
                            nc.tensor.matmul(
                                out=ps[:],
                                lhsT=mt[:, (m_off + k) * P:(m_off + k + 1) * P],
                                rhs=ylo[:, lo_off + cc, :],
                                start=(k == 0), stop=False)
                            k += 1
                        for cc in range(int(THI[pos])):
                            nc.tensor.matmul(
                                out=ps[:],
                                lhsT=mt[:, (m_off + k) * P:(m_off + k + 1) * P],
                                rhs=yhi[:, hi_off + cc, :],
                                start=(k == 0), stop=False)
                            k += 1
                        nc.tensor.matmul(
                            out=ps[:], lhsT=mbias_t[:], rhs=bias_t[:],
                            start=(k == 0), stop=True)
                        ot = oep.tile([P, FEAT], f32, tag="ot")
                        nc.scalar.activation(
                            out=ot[:], in_=ps[:],
                            func=mybir.ActivationFunctionType.Relu)
                        nc.vector.tensor_add(out=oacc[:], in0=oacc[:],
                                             in1=ot[:, 0:P])
                        nc.sync.dma_start(out=outp_d[pos * P:(pos + 1) * P, :],
                                          in_=ot[:])
                        lo_off += int(TLO[pos])
                        hi_off += int(THI[pos])
                        m_off += nchp

                sps_t = spsp.tile([1, P], f32)
                nc.tensor.matmul(out=sps_t[:], lhsT=ones_t[:], rhs=oacc[:],
                                 start=True, stop=True)
                st = oep.tile([1, P], f32, tag="st")
                nc.vector.tensor_copy(out=st[:], in_=sps_t[:])
                nc.sync.dma_start(out=summ_d[:], in_=st[:])

    nc.compile()
    return nc


def _unshard(prep, results):
    outs = []
    for bi in range(2):
        p_full = np.empty((N, P), np.float32)
        n_full = np.empty((N, P), np.float32)
        s_acc = np.zeros(P, np.float64)
        relu_b = np.maximum(prep["b"][bi], 0.0)
        for ci in range(CORES_PER_BRANCH):
            core = bi * CORES_PER_BRANCH + ci
            o = results[core]["outp"].reshape(TILES, P, FEAT)
            corelist = prep["orders"][core]
            valid = 0
            for pos, t in enumerate(corelist):
                r0 = t * P
                nrows = min(P, N - r0)
                valid += nrows
                p_full[r0:r0 + nrows] = o[pos, :nrows, 0:P]
                n_full[r0:r0 + nrows] = o[pos, :nrows, P:FEAT]
            s_acc += results[core]["summ"][0].astype(np.float64) \
                - (TILES * P - valid) * relu_b
        s = (s_acc / N).astype(np.float32).reshape(1, P)
        outs.extend([p_full, n_full, s])
    return (outs[0], outs[1], outs[2], outs[3], outs[4], outs[5])


_RUN_KWARGS = {}


def kernel(**inputs):
    prep = _prep(inputs)
    nc = _build(prep["TLO"], prep["THI"])
    res = run_bass_kernel_spmd(nc, prep["in_maps"], list(range(NCORES)),
                               **_RUN_KWARGS)
    out = _unshard(prep, res.results)
    kernel.last_result = res
    return out
